# revision 1
# baseline (speedup 1.0000x reference)
"""AttnPool segment-softmax kernel for 8 trn2 NeuronCores.

out[b,:] = sum_{i in seg b} softmax_b(tanh(x_i Wq + ctx_proj_b) . v) * x_i

Strategy: segment-aligned "supertiles" of PAD=2048 nodes (<=31 whole
segments + dummy slot 31 for padding). Softmax computed without the
max-subtraction (scores are bounded by ||v||_1 since |tanh|<=1, so exp
is safe in f32 and softmax is shift-invariant).

Host precomputes (cheap, vectorized): ctx_proj = ctx_vec @ Wk, the
supertile packing, x transposed (bf16, score matmul), x natural (bf16,
weighted-sum path), one-hot / one-hot-transposed local-segment
matrices (exact in bf16).

Device per supertile:
  qcT[d,n] = Wq.T @ xT  (+)  cp_local.T @ onehotT      (PSUM accum)
  hT = tanh(qcT)                                        (ACT)
  scores[n] = v . hT[:,n]   (per-subtile matmul, out [128,1] per col)
  ex = exp(scores)                                      (ACT)
  exx[n, 0:128] = ex_n * x_n ; exx[n,128] = ex_n        (DVE)
  seg[j, 0:129] += onehot.T @ exx                       (PSUM accum)
  out[j,:] = seg[j,0:128] * 1/(seg[j,128] + eps)        (DVE)
No collectives: cores own disjoint segment ranges. The supertile loop
is a Tile For_i with UNROLL supertiles per iteration; the back-edge
barrier resets semaphores so no instruction needs more than the
hardware's per-instruction sync-wait budget.
"""

import os
import sys

import numpy as np

sys.path.insert(0, "/opt/trn_rl_repo")

import ml_dtypes

N, D, C, B = 1_048_576, 128, 256, 16_384
NCORES = 8
PAD = 2048           # nodes per supertile
SMAX = 32            # local segment slots (31 real + 1 dummy)
NSUB = PAD // 128    # 16 subtiles of 128 nodes
UNROLL = 1           # supertiles per For_i iteration
O_XT = 0
O_XN = PAD
O_OHT = O_XN + NSUB * 128
O_OH = O_OHT + 1024
O_CP = O_OH + NSUB * SMAX
BLOB = O_CP + 128  # xT | xn | ohT | oh | cp
BF16 = ml_dtypes.bfloat16

LAST_EXEC_NS = None
LAST_PROFILE = None
LAST_T = None

_trace = bool(int(os.environ.get("KERNEL_TRACE", "0")))


def _pack_supertiles(seg_ids):
    """Greedy segment-aligned packing. Returns (seg0, nseg, node0, nnode) lists."""
    counts = np.bincount(seg_ids, minlength=B).astype(np.int64)
    offsets = np.zeros(B + 1, dtype=np.int64)
    np.cumsum(counts, out=offsets[1:])
    st = []
    cur_seg0 = 0
    cur_nseg = 0
    cur_nodes = 0
    for b in range(B):
        c = int(counts[b])
        assert c <= PAD, f"segment {b} has {c} nodes > PAD={PAD}"
        if cur_nseg + 1 > SMAX - 1 or cur_nodes + c > PAD:
            st.append((cur_seg0, cur_nseg, int(offsets[cur_seg0]), cur_nodes))
            cur_seg0 = b
            cur_nseg = 0
            cur_nodes = 0
        cur_nseg += 1
        cur_nodes += c
    st.append((cur_seg0, cur_nseg, int(offsets[cur_seg0]), cur_nodes))
    return st


def _build_program(T):
    import concourse.bacc as bacc
    import concourse.bass as bass
    import concourse.mybir as mybir
    from concourse.bass import ds
    from concourse.tile import TileContext

    f32 = mybir.dt.float32
    bf16 = mybir.dt.bfloat16
    AF = mybir.ActivationFunctionType

    nc = bacc.Bacc()
    # row-blocked layouts: every supertile owns 128 DRAM rows in each param
    blob_d = nc.declare_dram_parameter("blob", [T * 128, BLOB], bf16, isOutput=False)
    wq_d = nc.declare_dram_parameter("Wq", [128, 128], bf16, isOutput=False)
    v_d = nc.declare_dram_parameter("v", [128, 1], bf16, isOutput=False)
    out_d = nc.declare_dram_parameter("out", [T * 128, 128], f32, isOutput=True)

    with TileContext(nc) as tc:
        with (
            tc.tile_pool(name="const", bufs=1) as cpool,
            tc.tile_pool(name="blob", bufs=4) as blpool,
            tc.tile_pool(name="hT", bufs=2) as hpool,
            tc.tile_pool(name="exs", bufs=2) as expool,
            tc.tile_pool(name="outp", bufs=3) as opool,
            tc.tile_pool(name="qc", bufs=2, space="PSUM") as qcpool,
            tc.tile_pool(name="sc", bufs=2, space="PSUM") as scpool,
            tc.tile_pool(name="sg", bufs=2, space="PSUM") as sgpool,
        ):
            wq_sb = cpool.tile([128, 128], bf16)
            nc.sync.dma_start(out=wq_sb[:], in_=wq_d[:, :])
            v_sb = cpool.tile([128, 1], bf16)
            nc.sync.dma_start(out=v_sb[:], in_=v_d[:, :])

            if True:
                for t in range(T):
                    r = t * 128
                    blob = blpool.tile([128, BLOB], bf16, tag="blob")
                    for qq in range(4):
                        eng_d = nc.sync if qq % 2 == 0 else nc.scalar
                        c0 = qq * (BLOB // 4)
                        c1 = BLOB if qq == 3 else (qq + 1) * (BLOB // 4)
                        eng_d.dma_start(out=blob[:, c0:c1], in_=blob_d[ds(r, 128), c0:c1])

                    hT = hpool.tile([128, PAD], bf16, tag="hT")
                    ex = expool.tile([128, NSUB], f32, tag="ex")
                    exx = expool.tile([128, NSUB * 129], bf16, tag="exx")
                    sc = scpool.tile([128, NSUB], f32, tag="sc")

                    # qcT = Wq.T @ xT + cp_local.T @ onehotT, halves of 1024
                    for h in range(2):
                        qc = qcpool.tile([128, 1024], f32, tag="qc")
                        for k in range(2):
                            blk = 2 * h + k
                            nc.tensor.matmul(
                                qc[:, k * 512:(k + 1) * 512],
                                wq_sb[:],
                                blob[:, O_XT + blk * 512:O_XT + (blk + 1) * 512],
                                start=True, stop=False,
                            )
                            base = 64 * (blk // 2)
                            fo = O_OHT + (blk % 2) * 512
                            nc.tensor.matmul(
                                qc[:, k * 512:(k + 1) * 512],
                                blob[base:base + 32, O_CP:O_CP + 128],
                                blob[base:base + 32, fo:fo + 512],
                                start=False, stop=True,
                            )
                        nc.scalar.activation(
                            hT[:, h * 1024:(h + 1) * 1024], qc[:], AF.Tanh
                        )

                    # scores: one column per subtile
                    for s in range(NSUB):
                        nc.tensor.matmul(
                            sc[:, s:s + 1],
                            hT[:, s * 128:(s + 1) * 128],
                            v_sb[:],
                            start=True, stop=True,
                        )
                    nc.scalar.activation(ex[:], sc[:], AF.Exp)

                    # exx = [ex*x, ex]; segment-sum matmul accumulation
                    sg = sgpool.tile([SMAX, 129], f32, tag="sg")
                    for s in range(NSUB):
                        nc.vector.tensor_scalar_mul(
                            exx[:, s * 129:s * 129 + 128],
                            blob[:, O_XN + s * 128:O_XN + (s + 1) * 128],
                            ex[:, s:s + 1],
                        )
                        nc.vector.tensor_copy(
                            exx[:, s * 129 + 128:s * 129 + 129], ex[:, s:s + 1]
                        )
                        nc.tensor.matmul(
                            sg[:],
                            blob[:, O_OH + s * SMAX:O_OH + (s + 1) * SMAX],
                            exx[:, s * 129:(s + 1) * 129],
                            start=(s == 0), stop=(s == NSUB - 1),
                        )

                    # normalize: out = num / (den + eps)
                    den = opool.tile([SMAX, 1], f32, tag="den")
                    nc.vector.tensor_scalar_add(den[:], sg[:, 128:129], 1e-30)
                    rden = opool.tile([SMAX, 1], f32, tag="rden")
                    nc.vector.reciprocal(rden[:], den[:])
                    outp = opool.tile([SMAX, 128], f32, tag="outp")
                    nc.vector.tensor_scalar_mul(outp[:], sg[:, 0:128], rden[:])
                    nc.sync.dma_start(out=out_d[ds(r, SMAX)], in_=outp[:])

    nc.compile()
    return nc


def kernel(node_x, batch_idx, ctx_vec, Wq, Wk, v):
    global LAST_EXEC_NS, LAST_PROFILE
    node_x = np.ascontiguousarray(node_x, dtype=np.float32)
    seg_ids = np.asarray(batch_idx).astype(np.int32)
    ctx_vec = np.asarray(ctx_vec, dtype=np.float32)
    Wq = np.asarray(Wq, dtype=np.float32)
    Wk = np.asarray(Wk, dtype=np.float32)
    v = np.asarray(v, dtype=np.float32)

    cp = (ctx_vec @ Wk).astype(BF16)  # [B, 128]

    st = _pack_supertiles(seg_ids)
    nst = len(st)
    per = (nst + NCORES - 1) // NCORES
    T = ((per + UNROLL - 1) // UNROLL) * UNROLL

    blob_pk = np.zeros((NCORES, T * 128, BLOB), dtype=BF16)

    js = np.arange(SMAX, dtype=np.int32)
    for i, (seg0, nseg, node0, nn) in enumerate(st):
        c, t = divmod(i, T)
        r = t * 128
        xs = node_x[node0:node0 + nn]
        ls = np.full(PAD, SMAX - 1, dtype=np.int32)
        ls[:nn] = seg_ids[node0:node0 + nn] - seg0
        X = np.zeros((PAD, 128), dtype=np.float32)
        X[:nn] = xs
        Xb = X.astype(BF16)
        blob_pk[c, r:r + 128, O_XT:O_XT + PAD] = Xb.T
        blob_pk[c, r:r + 128, O_XN:O_XN + NSUB * 128] = (
            Xb.reshape(NSUB, 128, 128).transpose(1, 0, 2).reshape(128, NSUB * 128)
        )
        ohb = (ls[:, None] == js[None, :]).astype(BF16)  # [PAD, 32]
        ohTt = ohb.T  # [32, 2048]
        blob_pk[c, r:r + 32, O_OHT:O_OHT + 1024] = ohTt[:, 0:1024]
        blob_pk[c, r + 64:r + 96, O_OHT:O_OHT + 1024] = ohTt[:, 1024:2048]
        blob_pk[c, r:r + 128, O_OH:O_OH + NSUB * SMAX] = (
            ohb.reshape(NSUB, 128, SMAX).transpose(1, 0, 2).reshape(128, NSUB * SMAX)
        )
        for rr in (0, 64):  # replicate at the matmul base partitions
            blob_pk[c, r + rr:r + rr + nseg, O_CP:O_CP + 128] = cp[seg0:seg0 + nseg]

    global LAST_T
    LAST_T = T
    nc = _build_program(T)

    from concourse.bass_utils import run_bass_kernel_spmd

    in_maps = []
    for c in range(NCORES):
        in_maps.append({
            "blob": blob_pk[c],
            "Wq": Wq.astype(BF16),
            "v": v.reshape(128, 1).astype(BF16),
        })

    res = run_bass_kernel_spmd(nc, in_maps, list(range(NCORES)), trace=_trace)
    LAST_EXEC_NS = res.exec_time_ns
    LAST_PROFILE = res.profile_json

    out = np.zeros((B, 128), dtype=np.float32)
    for i, (seg0, nseg, node0, nn) in enumerate(st):
        c, t = divmod(i, T)
        out[seg0:seg0 + nseg] = res.results[c]["out"][t * 128:t * 128 + nseg]
    return out



# revision 12
# speedup vs baseline: 1.2936x; 1.2936x over previous
"""AttnPool segment-softmax kernel for 8 trn2 NeuronCores.

out[b,:] = sum_{i in seg b} softmax_b(tanh(x_i Wq + ctx_proj_b) . v) * x_i

Strategy: segment-aligned "supertiles" of PAD=2048 nodes (<=31 whole
segments + dummy slot 31 for padding). Softmax computed without the
max-subtraction (scores are bounded by ||v||_1 since |tanh|<=1, so exp
is safe in f32 and softmax is shift-invariant).

Host precomputes (cheap, vectorized): ctx_proj = ctx_vec @ Wk and the
per-supertile packed blob:
  xT   [128, 2048]  x transposed (bf16) - q matmul rhs
  xn   [128, 16*129] x natural per subtile + ones column (bf16) -
                     seg-sum matmul rhs (ones column -> denominator)
  ohT  [128, 512]   local-segment one-hot transposed, dense 4-band
                    packing (band b holds nodes b*512..b*512+512 at
                    partitions 32b..32b+32) - ctx matmul rhs
  cp   [128, 128]   ctx_proj rows for local segments, replicated at
                    all 4 partition bands - ctx matmul lhsT
  lb   [128, 16]    local-segment label per node (bf16 integer)

Device per supertile:
  qcT[d,n] = Wq.T @ xT  (+)  cp.T @ ohT     (PSUM accum, 4 bands)
  hT = tanh(qcT)                            (ACT)
  sc[n] = hT_s.T @ v   per subtile          (PE, 1 col each)
  ex = exp(sc)  bf16                        (ACT)
  ohw[n, j] = (iota==lb) * ex  per subtile  (DVE fused tensor_scalar)
  sg[j, 0:129] += ohw_s.T @ xn_s            (PSUM accum over subtiles)
  outp = copy(sg); DMA out                  (DVE copy, raw num|den)
Host divides num/den per segment (empty segments -> zero rows).

The loop is software-pipelined depth 3 (stages A=q/tanh, B=scores/ex/
ohw, C=seg-sum/out) so PE never waits for the ACT->DVE chain of the
same supertile. No collectives: cores own disjoint segment ranges.
"""

import os
import sys

import numpy as np

sys.path.insert(0, "/opt/trn_rl_repo")

import ml_dtypes

N, D, C, B = 1_048_576, 128, 256, 16_384
NCORES = 8
PAD = 2048           # nodes per supertile
SMAX = 32            # local segment slots (31 real + 1 dummy)
NSUB = PAD // 128    # 16 subtiles of 128 nodes
O_XT = 0
O_XN = PAD                    # 16*129 = 2064 cols
O_OHT = O_XN + NSUB * 129     # 512 cols
O_CP = O_OHT + 512            # 128 cols
O_LB = O_CP + 128             # 16 cols
BLOB = O_LB + NSUB            # 4768
BF16 = ml_dtypes.bfloat16

LAST_EXEC_NS = None
LAST_PROFILE = None
LAST_T = None

_trace = bool(int(os.environ.get("KERNEL_TRACE", "0")))


def _pack_supertiles(seg_ids, nsegs=B):
    """Greedy segment-aligned packing. Returns (seg0, nseg, node0, nnode)."""
    counts = np.bincount(seg_ids, minlength=nsegs).astype(np.int64)
    offsets = np.zeros(nsegs + 1, dtype=np.int64)
    np.cumsum(counts, out=offsets[1:])
    st = []
    cur_seg0 = 0
    cur_nseg = 0
    cur_nodes = 0
    for b in range(nsegs):
        c = int(counts[b])
        assert c <= PAD, f"segment {b} has {c} nodes > PAD={PAD}"
        if cur_nseg + 1 > SMAX or cur_nodes + c > PAD:
            st.append((cur_seg0, cur_nseg, int(offsets[cur_seg0]), cur_nodes))
            cur_seg0 = b
            cur_nseg = 0
            cur_nodes = 0
        cur_nseg += 1
        cur_nodes += c
    st.append((cur_seg0, cur_nseg, int(offsets[cur_seg0]), cur_nodes))
    return st


def _pack_blob(st, node_x, seg_ids, cp, ncores, T):
    """Build per-core packed blob arrays [ncores, T*128, BLOB] bf16."""
    blob_pk = np.zeros((ncores, T * 128, BLOB), dtype=BF16)
    js = np.arange(SMAX, dtype=np.int32)
    for i, (seg0, nseg, node0, nn) in enumerate(st):
        c, t = divmod(i, T)
        r = t * 128
        xs = node_x[node0:node0 + nn]
        # padding nodes: label 0 is fine - their x rows AND ones-column are
        # zero, so they contribute nothing to any slot's num or den
        ls = np.zeros(PAD, dtype=np.int32)
        ls[:nn] = seg_ids[node0:node0 + nn] - seg0
        X = np.zeros((PAD, 128), dtype=np.float32)
        X[:nn] = xs
        Xb = X.astype(BF16)
        blob_pk[c, r:r + 128, O_XT:O_XT + PAD] = Xb.T
        Xaug = np.zeros((PAD, 129), dtype=BF16)
        Xaug[:, :128] = Xb
        Xaug[:nn, 128] = BF16(1.0)
        blob_pk[c, r:r + 128, O_XN:O_XN + NSUB * 129] = (
            Xaug.reshape(NSUB, 128, 129).transpose(1, 0, 2).reshape(128, NSUB * 129)
        )
        ohT = (ls[None, :] == js[:, None]).astype(BF16)  # [32, 2048]
        for b in range(4):
            blob_pk[c, r + 32 * b:r + 32 * b + 32, O_OHT:O_OHT + 512] = (
                ohT[:, b * 512:(b + 1) * 512]
            )
            blob_pk[c, r + 32 * b:r + 32 * b + nseg, O_CP:O_CP + 128] = (
                cp[seg0:seg0 + nseg]
            )
        blob_pk[c, r:r + 128, O_LB:O_LB + NSUB] = (
            ls.astype(BF16).reshape(NSUB, 128).T
        )
    return blob_pk


def _build_program(T):
    import concourse.bacc as bacc
    import concourse.mybir as mybir
    from concourse.bass import ds
    from concourse.tile import TileContext

    f32 = mybir.dt.float32
    bf16 = mybir.dt.bfloat16
    AF = mybir.ActivationFunctionType
    ALU = mybir.AluOpType

    nc = bacc.Bacc()
    blob_d = nc.declare_dram_parameter("blob", [T * 128, BLOB], bf16, isOutput=False)
    wq_d = nc.declare_dram_parameter("Wq", [128, 128], bf16, isOutput=False)
    v_d = nc.declare_dram_parameter("v", [128, 1], bf16, isOutput=False)
    iota_d = nc.declare_dram_parameter("iota", [128, SMAX], bf16, isOutput=False)
    out_d = nc.declare_dram_parameter("out", [T * 32, 129], f32, isOutput=True)

    with TileContext(nc) as tc:
        with (
            tc.tile_pool(name="const", bufs=1) as cpool,
            tc.tile_pool(name="blob", bufs=8) as blpool,
            tc.tile_pool(name="hT", bufs=2) as hpool,
            tc.tile_pool(name="ex", bufs=2) as expool,
            tc.tile_pool(name="lbf", bufs=2) as lbpool,
            tc.tile_pool(name="ohw", bufs=2) as owpool,
            tc.tile_pool(name="outp", bufs=3) as opool,
            tc.tile_pool(name="qc", bufs=2, space="PSUM") as qcpool,
            tc.tile_pool(name="sc", bufs=2, space="PSUM") as scpool,
            tc.tile_pool(name="sg", bufs=2, space="PSUM") as sgpool,
        ):
            wq_sb = cpool.tile([128, 128], bf16)
            nc.sync.dma_start(out=wq_sb[:], in_=wq_d[:, :])
            v_sb = cpool.tile([128, 1], bf16)
            nc.sync.dma_start(out=v_sb[:], in_=v_d[:, :])
            iota_sb = cpool.tile([128, SMAX], bf16)
            nc.sync.dma_start(out=iota_sb[:], in_=iota_d[:, :])

            hist = {}  # t -> (blob, hT, ex, ohw)
            for t in range(T + 2):
                # ---- stage A: load, q+ctx matmuls, tanh -------------------
                if t < T:
                    r = t * 128
                    blob = blpool.tile([128, BLOB], bf16, tag="blob")
                    nc.sync.dma_start(out=blob[:], in_=blob_d[ds(r, 128), :])
                    hT = hpool.tile([128, PAD], bf16, tag="hT")
                    for h in range(2):
                        qc = qcpool.tile([128, 1024], f32, tag="qc")
                        for k in range(2):
                            blk = 2 * h + k
                            nc.tensor.matmul(
                                qc[:, k * 512:(k + 1) * 512],
                                wq_sb[:],
                                blob[:, O_XT + blk * 512:O_XT + (blk + 1) * 512],
                                start=True, stop=False,
                            )
                            p0 = 32 * blk
                            nc.tensor.matmul(
                                qc[:, k * 512:(k + 1) * 512],
                                blob[p0:p0 + 32, O_CP:O_CP + 128],
                                blob[p0:p0 + 32, O_OHT:O_OHT + 512],
                                start=False, stop=True,
                                tile_position=(p0, 0),
                            )
                        nc.scalar.activation(
                            hT[:, h * 1024:(h + 1) * 1024], qc[:], AF.Tanh
                        )
                    hist[t] = [blob, hT, None, None]

                # ---- stage B: scores, exp, weighted one-hot ---------------
                u = t - 1
                if 0 <= u < T:
                    blob_u, hT_u = hist[u][0], hist[u][1]
                    sc = scpool.tile([128, NSUB], f32, tag="sc")
                    for s in range(NSUB):
                        nc.tensor.matmul(
                            sc[:, s:s + 1],
                            hT_u[:, s * 128:(s + 1) * 128],
                            v_sb[:],
                            start=True, stop=True,
                        )
                    ex = expool.tile([128, NSUB], f32, tag="ex")
                    nc.scalar.activation(ex[:], sc[:], AF.Exp)
                    lbf = lbpool.tile([128, NSUB], f32, tag="lbf")
                    nc.vector.tensor_copy(lbf[:], blob_u[:, O_LB:O_LB + NSUB])
                    ohw = owpool.tile([128, NSUB * SMAX], bf16, tag="ohw")
                    for s in range(NSUB):
                        nc.vector.tensor_scalar(
                            ohw[:, s * SMAX:(s + 1) * SMAX],
                            iota_sb[:],
                            lbf[:, s:s + 1],
                            ex[:, s:s + 1],
                            ALU.is_equal,
                            ALU.mult,
                        )
                    hist[u][2] = ex
                    hist[u][3] = ohw

                # ---- stage C: segment sums, store -------------------------
                w = t - 2
                if w >= 0:
                    blob_w, ohw_w = hist[w][0], hist[w][3]
                    sg = sgpool.tile([SMAX, 129], f32, tag="sg")
                    for s in range(NSUB):
                        nc.tensor.matmul(
                            sg[:],
                            ohw_w[:, s * SMAX:(s + 1) * SMAX],
                            blob_w[:, O_XN + s * 129:O_XN + (s + 1) * 129],
                            start=(s == 0), stop=(s == NSUB - 1),
                        )
                    # batch 4 supertiles' results into one [128, 129] tile
                    # (4 partition bands) -> one store DMA per 4 iterations
                    g = w % 4
                    if g == 0:
                        outp = opool.tile([128, 129], f32, tag="outp")
                    nc.vector.tensor_copy(outp[32 * g:32 * (g + 1), :], sg[:])
                    if g == 3 or w == T - 1:
                        # separate queue from the blob stream: an in-order
                        # sync queue would stall blob DMAs behind this
                        # store's waits
                        nc.scalar.dma_start(
                            out=out_d[ds((w - g) * 32, 32 * (g + 1))],
                            in_=outp[0:32 * (g + 1), :],
                        )
                    del hist[w]

    nc.compile()
    return nc


def kernel(node_x, batch_idx, ctx_vec, Wq, Wk, v):
    global LAST_EXEC_NS, LAST_PROFILE, LAST_T
    node_x = np.ascontiguousarray(node_x, dtype=np.float32)
    seg_ids = np.asarray(batch_idx).astype(np.int32)
    ctx_vec = np.asarray(ctx_vec, dtype=np.float32)
    Wq = np.asarray(Wq, dtype=np.float32)
    Wk = np.asarray(Wk, dtype=np.float32)
    v = np.asarray(v, dtype=np.float32)

    cp = (ctx_vec @ Wk).astype(BF16)  # [B, 128]

    st = _pack_supertiles(seg_ids)
    nst = len(st)
    T = (nst + NCORES - 1) // NCORES

    blob_pk = _pack_blob(st, node_x, seg_ids, cp, NCORES, T)

    LAST_T = T
    nc = _build_program(T)

    from concourse.bass_utils import run_bass_kernel_spmd

    iota_np = np.broadcast_to(
        np.arange(SMAX, dtype=np.float32), (128, SMAX)
    ).astype(BF16)
    in_maps = []
    for c in range(NCORES):
        in_maps.append({
            "blob": blob_pk[c],
            "Wq": Wq.astype(BF16),
            "v": v.reshape(128, 1).astype(BF16),
            "iota": iota_np,
        })

    res = run_bass_kernel_spmd(nc, in_maps, list(range(NCORES)), trace=_trace)
    LAST_EXEC_NS = res.exec_time_ns
    LAST_PROFILE = res.profile_json

    out = np.zeros((B, 128), dtype=np.float32)
    for i, (seg0, nseg, node0, nn) in enumerate(st):
        c, t = divmod(i, T)
        raw = res.results[c]["out"][t * 32:t * 32 + nseg]  # [nseg, 129]
        den = raw[:, 128:129]
        num = raw[:, 0:128]
        nz = den[:, 0] != 0
        seg_out = np.zeros((nseg, 128), dtype=np.float32)
        seg_out[nz] = num[nz] / den[nz]
        out[seg0:seg0 + nseg] = seg_out
    return out


# revision 31
# speedup vs baseline: 1.3567x; 1.0488x over previous
"""AttnPool segment-softmax kernel for 8 trn2 NeuronCores.

out[b,:] = sum_{i in seg b} softmax_b(tanh(x_i Wq + ctx_proj_b) . v) * x_i

Strategy: segment-aligned "supertiles" of PAD=2048 nodes (<=31 whole
segments + dummy slot 31 for padding). Softmax computed without the
max-subtraction (scores are bounded by ||v||_1 since |tanh|<=1, so exp
is safe in f32 and softmax is shift-invariant).

Host precomputes (cheap, vectorized): ctx_proj = ctx_vec @ Wk and the
per-supertile packed blob:
  xT   [128, 2048]  x transposed (bf16) - q matmul rhs
  xn   [128, 16*129] x natural per subtile + ones column (bf16) -
                     seg-sum matmul rhs (ones column -> denominator)
  ohT  [128, 512]   local-segment one-hot transposed, dense 4-band
                    packing (band b holds nodes b*512..b*512+512 at
                    partitions 32b..32b+32) - ctx matmul rhs
  cp   [128, 128]   ctx_proj rows for local segments, replicated at
                    all 4 partition bands - ctx matmul lhsT
  lb   [128, 16]    local-segment label per node (bf16 integer)

Device per supertile:
  qcT[d,n] = Wq.T @ xT  (+)  cp.T @ ohT     (PSUM accum, 4 bands)
  hT = tanh(qcT)                            (ACT)
  sc[n] = hT_s.T @ v   per subtile          (PE, 1 col each)
  ex = exp(sc)  bf16                        (ACT)
  ohw[n, j] = (iota==lb) * ex  per subtile  (DVE fused tensor_scalar)
  sg[j, 0:129] += ohw_s.T @ xn_s            (PSUM accum over subtiles)
  outp = copy(sg); DMA out                  (DVE copy, raw num|den)
Host divides num/den per segment (empty segments -> zero rows).

The loop is software-pipelined depth 3 (stages A=q/tanh, B=scores/ex/
ohw, C=seg-sum/out) so PE never waits for the ACT->DVE chain of the
same supertile. No collectives: cores own disjoint segment ranges.
"""

import os
import sys

import numpy as np

sys.path.insert(0, "/opt/trn_rl_repo")

import ml_dtypes

N, D, C, B = 1_048_576, 128, 256, 16_384
NCORES = 8
PAD = 2048           # nodes per supertile
SMAX = 32            # local segment slots (31 real + 1 dummy)
NSUB = PAD // 128    # 16 subtiles of 128 nodes
O_XT = 0
O_XN = PAD                    # 16*129 = 2064 cols
O_CP = O_XN + NSUB * 129      # 128 cols
O_LB = O_CP + 128             # 16 cols
BLOB = O_LB + NSUB            # 4256
BF16 = ml_dtypes.bfloat16
FP8 = ml_dtypes.float8_e4m3   # one-hot entries 0/1 are exact in fp8

LAST_EXEC_NS = None
LAST_PROFILE = None
LAST_T = None

_trace = bool(int(os.environ.get("KERNEL_TRACE", "0")))


def _pack_supertiles(seg_ids, nsegs=B):
    """Greedy segment-aligned packing. Returns (seg0, nseg, node0, nnode)."""
    counts = np.bincount(seg_ids, minlength=nsegs).astype(np.int64)
    offsets = np.zeros(nsegs + 1, dtype=np.int64)
    np.cumsum(counts, out=offsets[1:])
    st = []
    cur_seg0 = 0
    cur_nseg = 0
    cur_nodes = 0
    for b in range(nsegs):
        c = int(counts[b])
        assert c <= PAD, f"segment {b} has {c} nodes > PAD={PAD}"
        if cur_nseg + 1 > SMAX or cur_nodes + c > PAD:
            st.append((cur_seg0, cur_nseg, int(offsets[cur_seg0]), cur_nodes))
            cur_seg0 = b
            cur_nseg = 0
            cur_nodes = 0
        cur_nseg += 1
        cur_nodes += c
    st.append((cur_seg0, cur_nseg, int(offsets[cur_seg0]), cur_nodes))
    return st


def _pack_blob(st, node_x, seg_ids, cp, ncores, T):
    """Build per-core packed blob [ncores,T*128,BLOB] bf16 + ohT fp8."""
    blob_pk = np.zeros((ncores, T * 128, BLOB), dtype=BF16)
    ohT_pk = np.zeros((ncores, T * 128, 512), dtype=FP8)
    js = np.arange(SMAX, dtype=np.int32)
    for i, (seg0, nseg, node0, nn) in enumerate(st):
        c, t = divmod(i, T)
        r = t * 128
        xs = node_x[node0:node0 + nn]
        # padding nodes: label 0 is fine - their x rows AND ones-column are
        # zero, so they contribute nothing to any slot's num or den
        ls = np.zeros(PAD, dtype=np.int32)
        ls[:nn] = seg_ids[node0:node0 + nn] - seg0
        X = np.zeros((PAD, 128), dtype=np.float32)
        X[:nn] = xs
        Xb = X.astype(BF16)
        blob_pk[c, r:r + 128, O_XT:O_XT + PAD] = Xb.T
        Xaug = np.zeros((PAD, 129), dtype=BF16)
        Xaug[:, :128] = Xb
        Xaug[:nn, 128] = BF16(1.0)
        blob_pk[c, r:r + 128, O_XN:O_XN + NSUB * 129] = (
            Xaug.reshape(NSUB, 128, 129).transpose(1, 0, 2).reshape(128, NSUB * 129)
        )
        ohT = (ls[None, :] == js[:, None]).astype(FP8)  # [32, 2048]
        for b in range(4):
            ohT_pk[c, r + 32 * b:r + 32 * b + 32, :] = ohT[:, b * 512:(b + 1) * 512]
            blob_pk[c, r + 32 * b:r + 32 * b + nseg, O_CP:O_CP + 128] = (
                cp[seg0:seg0 + nseg]
            )
        blob_pk[c, r:r + 128, O_LB:O_LB + NSUB] = (
            ls.astype(BF16).reshape(NSUB, 128).T
        )
    return blob_pk, ohT_pk


def _build_program(T):
    import concourse.bacc as bacc
    import concourse.mybir as mybir
    from concourse.bass import ds
    from concourse.tile import TileContext

    f32 = mybir.dt.float32
    bf16 = mybir.dt.bfloat16
    fp8 = mybir.dt.float8e4
    AF = mybir.ActivationFunctionType
    ALU = mybir.AluOpType

    nc = bacc.Bacc()
    blob_d = nc.declare_dram_parameter("blob", [T * 128, BLOB], bf16, isOutput=False)
    ohT_d = nc.declare_dram_parameter("ohT", [T * 128, 512], fp8, isOutput=False)
    # consts merged into one tensor: Wq | v | iota -> single startup DMA
    cst_d = nc.declare_dram_parameter(
        "cst", [128, 128 + 1 + SMAX], bf16, isOutput=False
    )
    out_d = nc.declare_dram_parameter("out", [T * 32, 129], f32, isOutput=True)

    with TileContext(nc) as tc:
        with (
            tc.tile_pool(name="const", bufs=1) as cpool,
            tc.tile_pool(name="blob", bufs=10) as blpool,
            tc.tile_pool(name="ohTp", bufs=10) as ohpool,
            tc.tile_pool(name="hT", bufs=4) as hpool,
            tc.tile_pool(name="ex", bufs=3) as expool,
            tc.tile_pool(name="lbf", bufs=3) as lbpool,
            tc.tile_pool(name="ohw", bufs=3) as owpool,
            tc.tile_pool(name="outp", bufs=3) as opool,
            tc.tile_pool(name="qc", bufs=2, space="PSUM") as qcpool,
            tc.tile_pool(name="sc", bufs=2, space="PSUM") as scpool,
            tc.tile_pool(name="sg", bufs=2, space="PSUM") as sgpool,
        ):
            # consts on the scalar queue so the first blob DMA (sync queue)
            # is not serialized behind them
            cst_sb = cpool.tile([128, 128 + 1 + SMAX], bf16)
            nc.scalar.dma_start(out=cst_sb[:], in_=cst_d[:, :])
            wq_sb = cst_sb[:, 0:128]
            v_sb = cst_sb[:, 128:129]
            iota_sb = cst_sb[:, 129:129 + SMAX]

            hist = {}  # t -> (blob, hT, lbf, ohw)
            for t in range(T + 2):
                u = t - 1
                # scores for u=t-1 first in the PE stream (they only need
                # hT_u, ready since last iteration)
                if 0 <= u < T:
                    hT_u = hist[u][1]
                    sc = scpool.tile([128, NSUB], f32, tag="sc")
                    for s in range(NSUB):
                        nc.tensor.matmul(
                            sc[:, s:s + 1],
                            hT_u[:, s * 128:(s + 1) * 128],
                            v_sb,
                            start=True, stop=True,
                        )

                # ---- stage A: load, q+ctx matmuls, tanh (2 halves) --------
                # exp_{t-1} is emitted BETWEEN the two tanh halves: putting
                # it first would close the cycle tanh_h1_t -> sc_t ->
                # exp_t -> tanh_h0_{t+1} on the in-order ACT queue and pace
                # the whole pipeline above the DMA floor
                blob = hT = None
                if t < T:
                    r = t * 128
                    blob = blpool.tile([128, BLOB], bf16, tag="blob")
                    nc.sync.dma_start(out=blob[:], in_=blob_d[ds(r, 128), :])
                    ohT = ohpool.tile([128, 512], fp8, tag="ohT")
                    nc.scalar.dma_start(out=ohT[:], in_=ohT_d[ds(r, 128), :])
                    hT = hpool.tile([128, PAD], bf16, tag="hT")

                    def half(h):
                        qc = qcpool.tile([128, 1024], f32, tag="qc")
                        for k in range(2):
                            blk = 2 * h + k
                            nc.tensor.matmul(
                                qc[:, k * 512:(k + 1) * 512],
                                wq_sb,
                                blob[:, O_XT + blk * 512:O_XT + (blk + 1) * 512],
                                start=True, stop=False,
                            )
                            p0 = 32 * blk
                            nc.tensor.matmul(
                                qc[:, k * 512:(k + 1) * 512],
                                blob[p0:p0 + 32, O_CP:O_CP + 128],
                                ohT[p0:p0 + 32, :],
                                start=False, stop=True,
                                tile_position=(p0, 0),
                            )
                        nc.scalar.activation(
                            hT[:, h * 1024:(h + 1) * 1024], qc[:], AF.Tanh
                        )

                    half(0)

                # stage B rest: exp + weighted one-hot for u
                if 0 <= u < T:
                    ex = expool.tile([128, NSUB], f32, tag="ex")
                    nc.scalar.activation(ex[:], sc[:], AF.Exp)
                    ohw = owpool.tile([128, NSUB * SMAX], bf16, tag="ohw")
                    for s in range(NSUB):
                        nc.vector.tensor_scalar(
                            ohw[:, s * SMAX:(s + 1) * SMAX],
                            iota_sb,
                            hist[u][2][:, s:s + 1],
                            ex[:, s:s + 1],
                            ALU.is_equal,
                            ALU.mult,
                        )
                    hist[u][3] = ohw

                if t < T:
                    half(1)
                    lbf = lbpool.tile([128, NSUB], f32, tag="lbf")
                    nc.vector.tensor_copy(lbf[:], blob[:, O_LB:O_LB + NSUB])
                    hist[t] = [blob, hT, lbf, None]

                # ---- stage C: segment sums, store -------------------------
                w = t - 2
                if w >= 0:
                    blob_w, ohw_w = hist[w][0], hist[w][3]
                    sg = sgpool.tile([SMAX, 129], f32, tag="sg")
                    for s in range(NSUB):
                        nc.tensor.matmul(
                            sg[:],
                            ohw_w[:, s * SMAX:(s + 1) * SMAX],
                            blob_w[:, O_XN + s * 129:O_XN + (s + 1) * 129],
                            start=(s == 0), stop=(s == NSUB - 1),
                        )
                    # batch 4 supertiles' results into one [128, 129] tile
                    # (4 partition bands) -> one store DMA per 4 iterations
                    g = w % 4
                    if g == 0:
                        outp = opool.tile([128, 129], f32, tag="outp")
                    nc.vector.tensor_copy(outp[32 * g:32 * (g + 1), :], sg[:])
                    if g == 3 or w == T - 1:
                        # third queue (Pool/SWDGE): this store waits on late
                        # stage-C data, so sharing a queue with the blob or
                        # ohT loads would stall those streams behind it
                        nc.gpsimd.dma_start(
                            out=out_d[ds((w - g) * 32, 32 * (g + 1))],
                            in_=outp[0:32 * (g + 1), :],
                        )
                    del hist[w]

    nc.compile()
    return nc


def kernel(node_x, batch_idx, ctx_vec, Wq, Wk, v):
    global LAST_EXEC_NS, LAST_PROFILE, LAST_T
    node_x = np.ascontiguousarray(node_x, dtype=np.float32)
    seg_ids = np.asarray(batch_idx).astype(np.int32)
    ctx_vec = np.asarray(ctx_vec, dtype=np.float32)
    Wq = np.asarray(Wq, dtype=np.float32)
    Wk = np.asarray(Wk, dtype=np.float32)
    v = np.asarray(v, dtype=np.float32)

    cp = (ctx_vec @ Wk).astype(BF16)  # [B, 128]

    st = _pack_supertiles(seg_ids)
    nst = len(st)
    T = (nst + NCORES - 1) // NCORES

    blob_pk, ohT_pk = _pack_blob(st, node_x, seg_ids, cp, NCORES, T)

    LAST_T = T
    nc = _build_program(T)

    from concourse.bass_utils import run_bass_kernel_spmd

    cst_np = np.zeros((128, 128 + 1 + SMAX), dtype=BF16)
    cst_np[:, 0:128] = Wq.astype(BF16)
    cst_np[:, 128] = v.astype(BF16)
    cst_np[:, 129:129 + SMAX] = np.arange(SMAX, dtype=np.float32).astype(BF16)
    in_maps = []
    for c in range(NCORES):
        in_maps.append({
            "blob": blob_pk[c],
            "ohT": ohT_pk[c],
            "cst": cst_np,
        })

    res = run_bass_kernel_spmd(nc, in_maps, list(range(NCORES)), trace=_trace)
    LAST_EXEC_NS = res.exec_time_ns
    LAST_PROFILE = res.profile_json

    out = np.zeros((B, 128), dtype=np.float32)
    for i, (seg0, nseg, node0, nn) in enumerate(st):
        c, t = divmod(i, T)
        raw = res.results[c]["out"][t * 32:t * 32 + nseg]  # [nseg, 129]
        den = raw[:, 128:129]
        num = raw[:, 0:128]
        nz = den[:, 0] != 0
        seg_out = np.zeros((nseg, 128), dtype=np.float32)
        seg_out[nz] = num[nz] / den[nz]
        out[seg0:seg0 + nseg] = seg_out
    return out


# revision 40
# speedup vs baseline: 1.5192x; 1.1197x over previous
"""AttnPool segment-softmax kernel for 8 trn2 NeuronCores.

out[b,:] = sum_{i in seg b} softmax_b(tanh(x_i Wq + ctx_proj_b) . v) * x_i

Strategy: segment-aligned "supertiles" of PAD=2048 nodes (<=31 whole
segments + dummy slot 31 for padding). Softmax computed without the
max-subtraction (scores are bounded by ||v||_1 since |tanh|<=1, so exp
is safe in f32 and softmax is shift-invariant).

Host precomputes (cheap, vectorized): ctx_proj = ctx_vec @ Wk and the
per-supertile packed blob:
  xT   [128, 2048]  x transposed (bf16) - q matmul rhs
  xn   [128, 16*129] x natural per subtile + ones column (bf16) -
                     seg-sum matmul rhs (ones column -> denominator)
  ohT  [128, 512]   local-segment one-hot transposed, dense 4-band
                    packing (band b holds nodes b*512..b*512+512 at
                    partitions 32b..32b+32) - ctx matmul rhs
  cp   [128, 128]   ctx_proj rows for local segments, replicated at
                    all 4 partition bands - ctx matmul lhsT
  lb   [128, 16]    local-segment label per node (bf16 integer)

Device per supertile:
  qcT[d,n] = Wq.T @ xT  (+)  cp.T @ ohT     (PSUM accum, 4 bands)
  hT = tanh(qcT)                            (ACT)
  sc[n] = hT_s.T @ v   per subtile          (PE, 1 col each)
  ex = exp(sc)  bf16                        (ACT)
  ohw[n, j] = (iota==lb) * ex  per subtile  (DVE fused tensor_scalar)
  sg[j, 0:129] += ohw_s.T @ xn_s            (PSUM accum over subtiles)
  outp = copy(sg); DMA out                  (DVE copy, raw num|den)
Host divides num/den per segment (empty segments -> zero rows).

The loop is software-pipelined depth 3 (stages A=q/tanh, B=scores/ex/
ohw, C=seg-sum/out) so PE never waits for the ACT->DVE chain of the
same supertile. No collectives: cores own disjoint segment ranges.
"""

import os
import sys

import numpy as np

sys.path.insert(0, "/opt/trn_rl_repo")

import ml_dtypes

N, D, C, B = 1_048_576, 128, 256, 16_384
NCORES = 8
PAD = 2048           # nodes per supertile
SMAX = 32            # local segment slots (31 real + 1 dummy)
NSUB = PAD // 128    # 16 subtiles of 128 nodes
NLOAD = 11           # subtiles whose natural-layout x is loaded from DRAM
NDEV = NSUB - NLOAD  # subtiles transposed on-device (PE transpose + DVE evac)
O_XT = 0
O_XN = PAD                    # NLOAD*129 cols
O_CP = O_XN + NLOAD * 129     # 128 cols
O_LB = O_CP + 128             # 16 cols
BLOB = O_LB + NSUB            # 3740
PAD_LABEL = 33.0     # label for padding nodes: >= SMAX -> one-hot all-zero
BF16 = ml_dtypes.bfloat16
FP8 = ml_dtypes.float8_e4m3   # one-hot entries 0/1 are exact in fp8

LAST_EXEC_NS = None
LAST_PROFILE = None
LAST_T = None

_trace = bool(int(os.environ.get("KERNEL_TRACE", "0")))


def _pack_supertiles(seg_ids, nsegs=B):
    """Greedy segment-aligned packing. Returns (seg0, nseg, node0, nnode)."""
    counts = np.bincount(seg_ids, minlength=nsegs).astype(np.int64)
    offsets = np.zeros(nsegs + 1, dtype=np.int64)
    np.cumsum(counts, out=offsets[1:])
    st = []
    cur_seg0 = 0
    cur_nseg = 0
    cur_nodes = 0
    for b in range(nsegs):
        c = int(counts[b])
        assert c <= PAD, f"segment {b} has {c} nodes > PAD={PAD}"
        if cur_nseg + 1 > SMAX or cur_nodes + c > PAD:
            st.append((cur_seg0, cur_nseg, int(offsets[cur_seg0]), cur_nodes))
            cur_seg0 = b
            cur_nseg = 0
            cur_nodes = 0
        cur_nseg += 1
        cur_nodes += c
    st.append((cur_seg0, cur_nseg, int(offsets[cur_seg0]), cur_nodes))
    return st


def _pack_blob(st, node_x, seg_ids, cp, ncores, T):
    """Build per-core packed blob [ncores,T*128,BLOB] bf16 + ohT fp8."""
    blob_pk = np.zeros((ncores, T * 128, BLOB), dtype=BF16)
    ohT_pk = np.zeros((ncores, T * 128, 512), dtype=FP8)
    js = np.arange(SMAX, dtype=np.int32)
    for i, (seg0, nseg, node0, nn) in enumerate(st):
        c, t = divmod(i, T)
        r = t * 128
        xs = node_x[node0:node0 + nn]
        # padding nodes: label >= SMAX makes their one-hot row all-zero, so
        # they contribute nothing to any slot's num or den
        ls = np.full(PAD, PAD_LABEL, dtype=np.float32)
        ls[:nn] = seg_ids[node0:node0 + nn] - seg0
        X = np.zeros((PAD, 128), dtype=np.float32)
        X[:nn] = xs
        Xb = X.astype(BF16)
        blob_pk[c, r:r + 128, O_XT:O_XT + PAD] = Xb.T
        Xaug = np.zeros((NLOAD * 128, 129), dtype=BF16)
        Xaug[:, :128] = Xb[:NLOAD * 128]
        Xaug[:min(nn, NLOAD * 128), 128] = BF16(1.0)
        blob_pk[c, r:r + 128, O_XN:O_XN + NLOAD * 129] = (
            Xaug.reshape(NLOAD, 128, 129).transpose(1, 0, 2).reshape(128, NLOAD * 129)
        )
        ohT = (ls[None, :] == js[:, None]).astype(FP8)  # [32, 2048]
        for b in range(4):
            ohT_pk[c, r + 32 * b:r + 32 * b + 32, :] = ohT[:, b * 512:(b + 1) * 512]
            blob_pk[c, r + 32 * b:r + 32 * b + nseg, O_CP:O_CP + 128] = (
                cp[seg0:seg0 + nseg]
            )
        blob_pk[c, r:r + 128, O_LB:O_LB + NSUB] = (
            ls.astype(BF16).reshape(NSUB, 128).T
        )
    return blob_pk, ohT_pk


def _build_program(T):
    import concourse.bacc as bacc
    import concourse.mybir as mybir
    from concourse.bass import ds
    from concourse.tile import TileContext

    f32 = mybir.dt.float32
    bf16 = mybir.dt.bfloat16
    fp8 = mybir.dt.float8e4
    AF = mybir.ActivationFunctionType
    ALU = mybir.AluOpType

    nc = bacc.Bacc()
    blob_d = nc.declare_dram_parameter("blob", [T * 128, BLOB], bf16, isOutput=False)
    ohT_d = nc.declare_dram_parameter("ohT", [T * 128, 512], fp8, isOutput=False)
    # consts in one tensor: Wq | v | iota | identity | ones -> one startup DMA
    CST = 128 + 1 + SMAX + 128 + 1
    cst_d = nc.declare_dram_parameter("cst", [128, CST], bf16, isOutput=False)
    out_d = nc.declare_dram_parameter("out", [T * 32, 129], f32, isOutput=True)

    with TileContext(nc) as tc:
        with (
            tc.tile_pool(name="const", bufs=1) as cpool,
            tc.tile_pool(name="blob", bufs=10) as blpool,
            tc.tile_pool(name="ohTp", bufs=10) as ohpool,
            tc.tile_pool(name="hT", bufs=4) as hpool,
            tc.tile_pool(name="ex", bufs=3) as expool,
            tc.tile_pool(name="lbf", bufs=3) as lbpool,
            tc.tile_pool(name="ohw", bufs=3) as owpool,
            tc.tile_pool(name="outp", bufs=3) as opool,
            tc.tile_pool(name="xnd", bufs=3) as xndpool,
            tc.tile_pool(name="qc", bufs=2, space="PSUM") as qcpool,
            tc.tile_pool(name="sc", bufs=1, space="PSUM") as scpool,
            tc.tile_pool(name="sg", bufs=1, space="PSUM") as sgpool,
            tc.tile_pool(name="xp", bufs=2, space="PSUM") as xppool,
        ):
            # consts on the scalar queue so the first blob DMA (sync queue)
            # is not serialized behind them
            cst_sb = cpool.tile([128, 128 + 1 + SMAX + 128 + 1], bf16)
            nc.scalar.dma_start(out=cst_sb[:], in_=cst_d[:, :])
            wq_sb = cst_sb[:, 0:128]
            v_sb = cst_sb[:, 128:129]
            iota_sb = cst_sb[:, 129:129 + SMAX]
            id_sb = cst_sb[:, 161:289]
            ones_sb = cst_sb[:, 289:290]

            hist = {}  # t -> (blob, hT, lbf, ohw)
            for t in range(T + 2):
                u = t - 1
                # scores for u=t-1 first in the PE stream (they only need
                # hT_u, ready since last iteration)
                if 0 <= u < T:
                    hT_u = hist[u][1]
                    sc = scpool.tile([128, NSUB], f32, tag="sc")
                    for s in range(NSUB):
                        nc.tensor.matmul(
                            sc[:, s:s + 1],
                            hT_u[:, s * 128:(s + 1) * 128],
                            v_sb,
                            start=True, stop=True,
                        )

                # ---- stage A: load, q+ctx matmuls, tanh (2 halves) --------
                # exp_{t-1} is emitted BETWEEN the two tanh halves: putting
                # it first would close the cycle tanh_h1_t -> sc_t ->
                # exp_t -> tanh_h0_{t+1} on the in-order ACT queue and pace
                # the whole pipeline above the DMA floor
                blob = hT = None
                if t < T:
                    r = t * 128
                    blob = blpool.tile([128, BLOB], bf16, tag="blob")
                    nc.sync.dma_start(out=blob[:], in_=blob_d[ds(r, 128), :])
                    ohT = ohpool.tile([128, 512], fp8, tag="ohT")
                    nc.scalar.dma_start(out=ohT[:], in_=ohT_d[ds(r, 128), :])
                    hT = hpool.tile([128, PAD], bf16, tag="hT")

                    def half(h):
                        qc = qcpool.tile([128, 1024], f32, tag="qc")
                        for k in range(2):
                            blk = 2 * h + k
                            nc.tensor.matmul(
                                qc[:, k * 512:(k + 1) * 512],
                                wq_sb,
                                blob[:, O_XT + blk * 512:O_XT + (blk + 1) * 512],
                                start=True, stop=False,
                            )
                            p0 = 32 * blk
                            nc.tensor.matmul(
                                qc[:, k * 512:(k + 1) * 512],
                                blob[p0:p0 + 32, O_CP:O_CP + 128],
                                ohT[p0:p0 + 32, :],
                                start=False, stop=True,
                                tile_position=(p0, 0),
                            )
                        nc.scalar.activation(
                            hT[:, h * 1024:(h + 1) * 1024], qc[:], AF.Tanh
                        )

                    half(0)

                # stage B rest: exp + weighted one-hot for u
                if 0 <= u < T:
                    ex = expool.tile([128, NSUB], f32, tag="ex")
                    nc.scalar.activation(ex[:], sc[:], AF.Exp)
                    ohw = owpool.tile([128, NSUB * SMAX], bf16, tag="ohw")
                    for s in range(NSUB):
                        nc.vector.tensor_scalar(
                            ohw[:, s * SMAX:(s + 1) * SMAX],
                            iota_sb,
                            hist[u][2][:, s:s + 1],
                            ex[:, s:s + 1],
                            ALU.is_equal,
                            ALU.mult,
                        )
                    hist[u][3] = ohw

                if t < T:
                    half(1)
                    # transpose the last NDEV subtiles' x on-device: cheaper
                    # on idle PE cycles than re-loading natural-layout x over
                    # the saturated DMA
                    xp = xppool.tile([128, NDEV * 128], bf16, tag="xp")
                    for j in range(NDEV):
                        s = NLOAD + j
                        nc.tensor.transpose(
                            xp[:, j * 128:(j + 1) * 128],
                            blob[:, O_XT + s * 128:O_XT + (s + 1) * 128],
                            id_sb,
                        )
                    xnd = xndpool.tile([128, NDEV * 128], bf16, tag="xnd")
                    nc.vector.tensor_copy(xnd[:], xp[:])
                    lbf = lbpool.tile([128, NSUB], f32, tag="lbf")
                    nc.vector.tensor_copy(lbf[:], blob[:, O_LB:O_LB + NSUB])
                    hist[t] = [blob, hT, lbf, None, xnd]

                # ---- stage C: segment sums, store -------------------------
                w = t - 2
                if w >= 0:
                    blob_w, ohw_w, xnd_w = hist[w][0], hist[w][3], hist[w][4]
                    sg = sgpool.tile([SMAX, 129], f32, tag="sg")
                    for s in range(NLOAD):
                        nc.tensor.matmul(
                            sg[:],
                            ohw_w[:, s * SMAX:(s + 1) * SMAX],
                            blob_w[:, O_XN + s * 129:O_XN + (s + 1) * 129],
                            start=(s == 0), stop=False,
                        )
                    for j in range(NDEV):
                        s = NLOAD + j
                        ohw_s = ohw_w[:, s * SMAX:(s + 1) * SMAX]
                        nc.tensor.matmul(
                            sg[:, 0:128],
                            ohw_s,
                            xnd_w[:, j * 128:(j + 1) * 128],
                            start=False, stop=False,
                        )
                        nc.tensor.matmul(
                            sg[:, 128:129],
                            ohw_s,
                            ones_sb,
                            start=False, stop=(j == NDEV - 1),
                        )
                    # batch 4 supertiles' results into one [128, 129] tile
                    # (4 partition bands) -> one store DMA per 4 iterations
                    g = w % 4
                    if g == 0:
                        outp = opool.tile([128, 129], f32, tag="outp")
                    nc.vector.tensor_copy(outp[32 * g:32 * (g + 1), :], sg[:])
                    if g == 3 or w == T - 1:
                        # third queue (Pool/SWDGE): this store waits on late
                        # stage-C data, so sharing a queue with the blob or
                        # ohT loads would stall those streams behind it
                        nc.gpsimd.dma_start(
                            out=out_d[ds((w - g) * 32, 32 * (g + 1))],
                            in_=outp[0:32 * (g + 1), :],
                        )
                    del hist[w]

    nc.compile()
    return nc


def kernel(node_x, batch_idx, ctx_vec, Wq, Wk, v):
    global LAST_EXEC_NS, LAST_PROFILE, LAST_T
    node_x = np.ascontiguousarray(node_x, dtype=np.float32)
    seg_ids = np.asarray(batch_idx).astype(np.int32)
    ctx_vec = np.asarray(ctx_vec, dtype=np.float32)
    Wq = np.asarray(Wq, dtype=np.float32)
    Wk = np.asarray(Wk, dtype=np.float32)
    v = np.asarray(v, dtype=np.float32)

    cp = (ctx_vec @ Wk).astype(BF16)  # [B, 128]

    st = _pack_supertiles(seg_ids)
    nst = len(st)
    T = (nst + NCORES - 1) // NCORES

    blob_pk, ohT_pk = _pack_blob(st, node_x, seg_ids, cp, NCORES, T)

    LAST_T = T
    nc = _build_program(T)

    from concourse.bass_utils import run_bass_kernel_spmd

    cst_np = np.zeros((128, 128 + 1 + SMAX + 128 + 1), dtype=BF16)
    cst_np[:, 0:128] = Wq.astype(BF16)
    cst_np[:, 128] = v.astype(BF16)
    cst_np[:, 129:129 + SMAX] = np.arange(SMAX, dtype=np.float32).astype(BF16)
    cst_np[:, 161:289] = np.eye(128, dtype=np.float32).astype(BF16)
    cst_np[:, 289] = BF16(1.0)
    in_maps = []
    for c in range(NCORES):
        in_maps.append({
            "blob": blob_pk[c],
            "ohT": ohT_pk[c],
            "cst": cst_np,
        })

    res = None
    for attempt in range(3):
        try:
            res = run_bass_kernel_spmd(
                nc, in_maps, list(range(NCORES)), trace=_trace
            )
            break
        except Exception:
            # transient NRT_EXEC_UNIT_UNRECOVERABLE faults have been seen on
            # this fabric; identical re-runs succeed
            if attempt == 2:
                raise
    LAST_EXEC_NS = res.exec_time_ns
    LAST_PROFILE = res.profile_json

    out = np.zeros((B, 128), dtype=np.float32)
    for i, (seg0, nseg, node0, nn) in enumerate(st):
        c, t = divmod(i, T)
        raw = res.results[c]["out"][t * 32:t * 32 + nseg]  # [nseg, 129]
        den = raw[:, 128:129]
        num = raw[:, 0:128]
        nz = den[:, 0] != 0
        seg_out = np.zeros((nseg, 128), dtype=np.float32)
        seg_out[nz] = num[nz] / den[nz]
        out[seg0:seg0 + nseg] = seg_out
    return out
